# revision 1
# baseline (speedup 1.0000x reference)
"""HGRNBitMLP (BitNet-style SwiGLU MLP) on 8 TRN2 NeuronCores.

Data-parallel over the 4096 tokens (512/core). Weight ternarization is
sharded 1/8 per core and the ternary weights (exact in bf16) are
AllGathered; the global mean(|w|) comes from a tiny AllReduce.
Activations are quantized to the int8 grid (exact in bf16), so every
matmul is an exact-integer bf16 matmul with f32 PSUM accumulation.
Per-token scales are applied outside the matmuls.

Layouts: x is loaded [tok, h], quantized, PE-transposed to xqT [h, tok].
mm1 produces y^T tiles [o, tok]; SwiGLU keeps h as [I, tok] (spilled to
DRAM) so mm2's operand q2T [I, tok] needs no transpose. mm2 computes
out [tok, H] directly.
"""
import sys

try:
    import concourse  # noqa: F401
except ImportError:
    sys.path.insert(0, "/opt/trn_rl_repo")

import numpy as np

import concourse.tile as tile
from concourse import bacc, mybir
from concourse.bass_utils import run_bass_kernel_spmd
from concourse.masks import make_identity

F32, BF16 = mybir.dt.float32, mybir.dt.bfloat16
Alu = mybir.AluOpType
Act = mybir.ActivationFunctionType
X = mybir.AxisListType.X

NC_N = 8
B, S, H, I = 2, 2048, 2048, 8192
O2 = 2 * I
TOK = B * S
TPC = TOK // NC_N   # 512 tokens/core
TT = TPC // 128     # 4 token tiles
HK = H // 128       # 16 h tiles
IK = I // 128       # 64 I tiles
GSH = H // NC_N     # 256 rows of w_gate^T per core
DSH = I // NC_N     # 1024 rows of w_down^T per core
EPS = 1e-5
C_MAGIC = 12582912.0  # 1.5*2^23; (x+C)-C rounds f32 to nearest-even int
CCHUNK = 2048


def build(nc):
    x_ap = nc.dram_tensor("x", [TPC, H], F32, kind="ExternalInput").ap()
    wg_ap = nc.dram_tensor("wgt", [GSH, O2], F32, kind="ExternalInput").ap()
    wd_ap = nc.dram_tensor("wdt", [DSH, H], F32, kind="ExternalInput").ap()
    gg_ap = nc.dram_tensor("gg", [1, H], F32, kind="ExternalInput").ap()
    gd_ap = nc.dram_tensor("gdc", [128, IK], F32, kind="ExternalInput").ap()
    y_ap = nc.dram_tensor("y", [TPC, H], F32, kind="ExternalOutput").ap()
    rg = [list(range(NC_N))]

    with tile.TileContext(nc) as tc:
        with tc.tile_pool(name="dram", bufs=1, space="DRAM") as dram, \
             tc.tile_pool(name="perm", bufs=1) as cp, \
             tc.tile_pool(name="colp", bufs=1) as colp:

            ident_b = cp.tile([128, 128], BF16)
            make_identity(nc, ident_b[:])
            ident_f = cp.tile([128, 128], F32)
            make_identity(nc, ident_f[:])
            ones = cp.tile([128, 1], F32)
            nc.gpsimd.memset(ones[:], 1.0)
            epsb = cp.tile([128, 1], F32)
            nc.gpsimd.memset(epsb[:], EPS)
            gdc_sb = cp.tile([128, IK], F32)
            nc.sync.dma_start(gdc_sb[:], gd_ap[:])

            # ---- P0w: sharded abs-sums + AllReduce ----
            with tc.tile_pool(name="statp", bufs=2) as stp:
                parts_g = colp.tile([128, 16], F32)
                for blk in range(2):
                    for ck in range(O2 // CCHUNK):
                        wch = stp.tile([128, CCHUNK], F32, tag="wch",
                                       name=f"wg{blk}_{ck}")
                        nc.sync.dma_start(
                            wch[:], wg_ap[blk * 128:(blk + 1) * 128,
                                          ck * CCHUNK:(ck + 1) * CCHUNK])
                        nc.vector.tensor_reduce(
                            parts_g[:, blk * 8 + ck:blk * 8 + ck + 1], wch[:],
                            axis=X, op=Alu.add, apply_absolute_value=True)
                parts_d = colp.tile([128, 8], F32)
                for blk in range(8):
                    wch = stp.tile([128, CCHUNK], F32, tag="wch",
                                   name=f"wd{blk}")
                    nc.sync.dma_start(wch[:], wd_ap[blk * 128:(blk + 1) * 128, :])
                    nc.vector.tensor_reduce(parts_d[:, blk:blk + 1], wch[:],
                                            axis=X, op=Alu.add,
                                            apply_absolute_value=True)
                sum_g = colp.tile([128, 1], F32)
                nc.vector.tensor_reduce(sum_g[:], parts_g[:], axis=X, op=Alu.add)
                sum_d = colp.tile([128, 1], F32)
                nc.vector.tensor_reduce(sum_d[:], parts_d[:], axis=X, op=Alu.add)
                stat_sb = colp.tile([1, 2], F32)
                with tc.tile_pool(name="psStat", bufs=1, space="PSUM") as psS:
                  for k, sv in enumerate((sum_g, sum_d)):
                    ps = psS.tile([1, 1], F32, tag="statps", name=f"sps{k}")
                    nc.tensor.matmul(ps[:], sv[:], ones[:], start=True, stop=True)
                    nc.scalar.copy(stat_sb[:, k:k + 1], ps[:])
                stat_in = dram.tile([1, 2], F32)
                stat_out = dram.tile([1, 2], F32, addr_space="Shared")
                nc.sync.dma_start(stat_in[:], stat_sb[:])
                nc.gpsimd.collective_compute("AllReduce", Alu.add,
                                             replica_groups=rg,
                                             ins=[stat_in[:]], outs=[stat_out[:]])
                stat_res = colp.tile([1, 2], F32)
                nc.sync.dma_start(stat_res[:], stat_out[:])

            def bcast_scaled(src, scale, name):
                t1 = colp.tile([1, 1], F32, name=f"{name}_s")
                nc.vector.tensor_scalar_mul(t1[:], src, scale)
                t2 = colp.tile([128, 1], F32, name=f"{name}_b")
                nc.gpsimd.partition_broadcast(t2[:], t1[:])
                return t2

            thr_g = bcast_scaled(stat_res[0:1, 0:1], 2.0 ** -26, "thrg")
            m_g = bcast_scaled(stat_res[0:1, 0:1], 2.0 ** -25, "mg")
            thr_d = bcast_scaled(stat_res[0:1, 1:2], 2.0 ** -25, "thrd")
            m_d = bcast_scaled(stat_res[0:1, 1:2], 2.0 ** -24, "md")
            nthr_g = colp.tile([128, 1], F32)
            nc.vector.tensor_scalar_mul(nthr_g[:], thr_g[:], -1.0)
            nthr_d = colp.tile([128, 1], F32)
            nc.vector.tensor_scalar_mul(nthr_d[:], thr_d[:], -1.0)

            # ---- P2: ternarize shards + chunked AllGathers ----
            NCH = 4  # o-chunks of 4096 for the gate AG
            tg_shs = [dram.tile([GSH, 4096], BF16, name=f"tgsh{k}")
                      for k in range(NCH)]
            tg_fulls = [dram.tile([H, 4096], BF16, addr_space="Shared",
                                  name=f"tgf{k}") for k in range(NCH)]
            td_sh = dram.tile([DSH, H], BF16)
            td_full = dram.tile([I, H], BF16, addr_space="Shared")

            with tc.tile_pool(name="ternp", bufs=2) as tp:
                def tern_cols(w_ap, blk, src_cs, thr, nthr, dst, dst_cs, nm):
                    w = tp.tile([128, CCHUNK], F32, tag="tw", name=f"tw{nm}")
                    nc.sync.dma_start(
                        w[:], w_ap[blk * 128:(blk + 1) * 128, src_cs])
                    a = tp.tile([128, CCHUNK], BF16, tag="ta", name=f"ta{nm}")
                    nc.vector.tensor_scalar(a[:], w[:], thr[:], 0.5,
                                            Alu.is_gt, Alu.subtract)
                    b = tp.tile([128, CCHUNK], BF16, tag="tb", name=f"tb{nm}")
                    nc.vector.tensor_scalar(b[:], w[:], nthr[:], 0.5,
                                            Alu.is_ge, Alu.subtract)
                    t = tp.tile([128, CCHUNK], BF16, tag="tc", name=f"tc{nm}")
                    nc.vector.tensor_tensor(t[:], a[:], b[:], Alu.add)
                    nc.sync.dma_start(dst[blk * 128:(blk + 1) * 128, dst_cs],
                                      t[:])

                # gate chunks in consumption order: k0 (gate lo), k2 (v lo),
                # k1 (gate hi), k3 (v hi); AG fires right after each chunk
                for k in (0, 2, 1, 3):
                    for blk in range(GSH // 128):
                        for sub in range(4096 // CCHUNK):
                            tern_cols(
                                wg_ap, blk,
                                slice(k * 4096 + sub * CCHUNK,
                                      k * 4096 + (sub + 1) * CCHUNK),
                                thr_g, nthr_g, tg_shs[k],
                                slice(sub * CCHUNK, (sub + 1) * CCHUNK),
                                f"g{k}_{blk}_{sub}")
                    nc.gpsimd.collective_compute(
                        "AllGather", Alu.bypass, replica_groups=rg,
                        ins=[tg_shs[k][:]], outs=[tg_fulls[k][:]])
                for blk in range(DSH // 128):
                    tern_cols(wd_ap, blk, slice(0, H), thr_d, nthr_d,
                              td_sh, slice(0, H), f"d{blk}")
                nc.gpsimd.collective_compute("AllGather", Alu.bypass,
                                             replica_groups=rg,
                                             ins=[td_sh[:]], outs=[td_full[:]])

            # ---- P0x: x rmsnorm + int8-grid quant + transpose ----
            xq_p = tc.tile_pool(name="xqp", bufs=1)
            xq_pool = xq_p.__enter__()
            gg_sb = xq_pool.tile([1, H], F32)
            nc.sync.dma_start(gg_sb[:], gg_ap[:])
            g_bc = xq_pool.tile([128, H], F32)
            nc.gpsimd.partition_broadcast(g_bc[:], gg_sb[:])
            xqT = xq_pool.tile([128, HK * TPC], BF16)
            amax1 = colp.tile([128, TT], F32)
            with tc.tile_pool(name="xwork", bufs=2) as xw, \
                 tc.tile_pool(name="psX", bufs=2, space="PSUM") as psX:
                for t in range(TT):
                    xt = xw.tile([128, H], F32, tag="xt", name=f"xt{t}")
                    nc.sync.dma_start(xt[:], x_ap[t * 128:(t + 1) * 128, :])
                    xsq = xw.tile([128, H], F32, tag="xsq", name=f"xsq{t}")
                    ssq = colp.tile([128, 1], F32, name=f"ssq{t}")
                    nc.scalar.activation(xsq[:], xt[:], Act.Square,
                                         accum_out=ssq[:])
                    sd = colp.tile([128, 1], F32, name=f"sd{t}")
                    nc.scalar.activation(sd[:], ssq[:], Act.Sqrt, bias=epsb[:],
                                         scale=1.0 / H)
                    rstd = colp.tile([128, 1], F32, name=f"rstd{t}")
                    nc.vector.reciprocal(rstd[:], sd[:])
                    xn = xw.tile([128, H], F32, tag="xn", name=f"xn{t}")
                    nc.vector.tensor_tensor(xn[:], xt[:], g_bc[:], Alu.mult)
                    nc.vector.tensor_scalar_mul(xn[:], xn[:], rstd[:])
                    am = amax1[:, t:t + 1]
                    nc.vector.tensor_reduce(am, xn[:], axis=X, op=Alu.max,
                                            apply_absolute_value=True)
                    nc.vector.tensor_scalar_max(am, am, EPS)
                    rc = colp.tile([128, 1], F32, name=f"rc{t}")
                    nc.vector.reciprocal(rc[:], am)
                    s1 = colp.tile([128, 1], F32, name=f"s1{t}")
                    nc.vector.tensor_scalar_mul(s1[:], rc[:], 127.0)
                    nc.vector.tensor_scalar(xn[:], xn[:], s1[:], C_MAGIC,
                                            Alu.mult, Alu.add)
                    q = xw.tile([128, H], BF16, tag="q", name=f"q{t}")
                    nc.vector.tensor_scalar(q[:], xn[:], C_MAGIC, None,
                                            Alu.subtract)
                    for i in range(HK):
                        tps = psX.tile([128, 128], BF16, tag="tps",
                                       name=f"tps{t}_{i}")
                        nc.tensor.transpose(tps[:], q[:, i * 128:(i + 1) * 128],
                                            ident_b[:])
                        nc.scalar.copy(xqT[:, i * TPC + t * 128:
                                           i * TPC + (t + 1) * 128], tps[:])

            ys_cols = colp.tile([128, TT], F32)
            nc.vector.tensor_scalar(ys_cols[:], amax1[:], m_g[:], 1.0 / 127.0,
                                    Alu.mult, Alu.mult)

            def cols_to_row_bcast(cols, name):
                with tc.tile_pool(name=f"psR{name}", bufs=1,
                                  space="PSUM") as psR:
                    ps = psR.tile([TT, 128], F32, tag="rowps",
                                  name=f"{name}_ps")
                    nc.tensor.transpose(ps[:], cols[:], ident_f[:])
                    r4 = colp.tile([TT, 128], F32, name=f"{name}_r4")
                    nc.scalar.copy(r4[:], ps[:])
                # bounce [4,128] -> [1,512] through DRAM (linear reinterp)
                rb = dram.tile([TT, 128], F32, name=f"{name}_rb")
                nc.sync.dma_start(rb[:], r4[:])
                row = colp.tile([1, TPC], F32, name=f"{name}_r")
                nc.sync.dma_start(row[:],
                                  rb[:].rearrange("a b -> (a b)").rearrange("(o f) -> o f", o=1))
                bc = colp.tile([128, TPC], F32, name=f"{name}_bc")
                nc.gpsimd.partition_broadcast(bc[:], row[:])
                return bc

            ys_bc = cols_to_row_bcast(ys_cols, "ys")

            # ---- P3: mm1 + SwiGLU -> h [I, tok] spilled to DRAM ----
            h_dram = dram.tile([I, TPC], F32)
            acc_sq = colp.tile([128, TPC], F32)
            nc.gpsimd.memset(acc_sq[:], 0.0)
            acc_mxp = colp.tile([128, TPC], F32)
            nc.gpsimd.memset(acc_mxp[:], -3.0e38)
            acc_mxn = colp.tile([128, TPC], F32)
            nc.gpsimd.memset(acc_mxn[:], 3.0e38)
            tgvs = [t[:].rearrange("(i p) o -> p i o", p=128)
                    for t in tg_fulls]

            with tc.tile_pool(name="p3", bufs=2) as p3, \
                 tc.tile_pool(name="psMM1", bufs=2, space="PSUM") as psM1:
              for sb in range(8):  # o-blocks of 1024 cols per half
                kg, og = sb // 4, (sb % 4) * 1024
                kv, ov = 2 + sb // 4, (sb % 4) * 1024
                tg_g = p3.tile([128, HK, 1024], BF16, tag="tg_g",
                               name=f"tgg{sb}")
                nc.sync.dma_start(tg_g[:], tgvs[kg][:, :, og:og + 1024])
                tg_v = p3.tile([128, HK, 1024], BF16, tag="tg_v",
                               name=f"tgv{sb}")
                nc.sync.dma_start(tg_v[:], tgvs[kv][:, :, ov:ov + 1024])
                for si in range(4):
                    s = sb * 4 + si
                    pg = [psM1.tile([128, TPC], F32, tag=f"pg{jj}",
                                    name=f"pg{s}_{jj}") for jj in range(2)]
                    pv = [psM1.tile([128, TPC], F32, tag=f"pv{jj}",
                                    name=f"pv{s}_{jj}") for jj in range(2)]
                    for i in range(HK):
                        rhs = xqT[:, i * TPC:(i + 1) * TPC]
                        st, sp = i == 0, i == HK - 1
                        for jj in range(2):
                            co = si * 256 + jj * 128
                            nc.tensor.matmul(
                                pg[jj][:], tg_g[:, i, co:co + 128],
                                rhs, start=st, stop=sp)
                            nc.tensor.matmul(
                                pv[jj][:], tg_v[:, i, co:co + 128],
                                rhs, start=st, stop=sp)
                    for jj in range(2):
                        j = 2 * s + jj
                        gsc = p3.tile([128, TPC], F32, tag="gsc", name=f"gs{j}")
                        nc.vector.tensor_tensor(gsc[:], pg[jj][:], ys_bc[:],
                                                Alu.mult)
                        sg = p3.tile([128, TPC], F32, tag="sg", name=f"sg{j}")
                        nc.scalar.activation(sg[:], gsc[:], Act.Silu)
                        vsc = p3.tile([128, TPC], F32, tag="vsc", name=f"vs{j}")
                        nc.vector.tensor_tensor(vsc[:], pv[jj][:], ys_bc[:],
                                                Alu.mult)
                        hj = p3.tile([128, TPC], F32, tag="hj", name=f"hj{j}")
                        nc.vector.tensor_tensor(hj[:], sg[:], vsc[:], Alu.mult)
                        nc.sync.dma_start(h_dram[j * 128:(j + 1) * 128, :],
                                          hj[:])
                        hsq = p3.tile([128, TPC], F32, tag="hsq", name=f"hq{j}")
                        nc.gpsimd.tensor_tensor(hsq[:], hj[:], hj[:], Alu.mult)
                        nc.gpsimd.tensor_tensor(acc_sq[:], acc_sq[:], hsq[:],
                                                Alu.add)
                        hg = p3.tile([128, TPC], F32, tag="hg", name=f"hg{j}")
                        nc.vector.tensor_scalar_mul(hg[:], hj[:],
                                                    gdc_sb[:, j:j + 1])
                        nc.vector.tensor_tensor(acc_mxp[:], acc_mxp[:], hg[:],
                                                Alu.max)
                        nc.vector.tensor_tensor(acc_mxn[:], acc_mxn[:], hg[:],
                                                Alu.min)

            xq_p.__exit__(None, None, None)

            # ---- P4: per-token stats over I ----
            qs_cols = colp.tile([128, TT], F32)
            y2s_cols = colp.tile([128, TT], F32)
            ps4_ctx = tc.tile_pool(name="ps4", bufs=1, space="PSUM")
            ps4 = ps4_ctx.__enter__()
            for t in range(TT):
                sl = slice(t * 128, (t + 1) * 128)
                pssq = ps4.tile([128, 128], F32, tag="pssq", name=f"pq{t}")
                nc.tensor.transpose(pssq[:], acc_sq[:, sl], ident_f[:])
                ss = colp.tile([128, 1], F32, name=f"hss{t}")
                nc.vector.tensor_reduce(ss[:], pssq[:], axis=X, op=Alu.add)
                psm1 = ps4.tile([128, 128], F32, tag="psm1", name=f"pm1{t}")
                nc.tensor.transpose(psm1[:], acc_mxp[:, sl], ident_f[:])
                c1 = colp.tile([128, 1], F32, name=f"hc1{t}")
                nc.vector.tensor_reduce(c1[:], psm1[:], axis=X, op=Alu.max,
                                        apply_absolute_value=True)
                psm2 = ps4.tile([128, 128], F32, tag="psm2", name=f"pm2{t}")
                nc.tensor.transpose(psm2[:], acc_mxn[:, sl], ident_f[:])
                c2 = colp.tile([128, 1], F32, name=f"hc2{t}")
                nc.vector.tensor_reduce(c2[:], psm2[:], axis=X, op=Alu.max,
                                        apply_absolute_value=True)
                amax_hg = colp.tile([128, 1], F32, name=f"amhg{t}")
                nc.vector.tensor_tensor(amax_hg[:], c1[:], c2[:], Alu.max)
                sd2 = colp.tile([128, 1], F32, name=f"sd2{t}")
                nc.scalar.activation(sd2[:], ss[:], Act.Sqrt, bias=epsb[:],
                                     scale=1.0 / I)
                rstd2 = colp.tile([128, 1], F32, name=f"rstd2{t}")
                nc.vector.reciprocal(rstd2[:], sd2[:])
                t1 = colp.tile([128, 1], F32, name=f"t1{t}")
                nc.vector.tensor_scalar(t1[:], amax_hg[:], rstd2[:], EPS,
                                        Alu.mult, Alu.max)
                rc2 = colp.tile([128, 1], F32, name=f"rc2{t}")
                nc.vector.reciprocal(rc2[:], t1[:])
                s2 = colp.tile([128, 1], F32, name=f"s2{t}")
                nc.vector.tensor_scalar_mul(s2[:], rc2[:], 127.0)
                nc.vector.tensor_scalar_mul(qs_cols[:, t:t + 1], rstd2[:],
                                            s2[:])
                nc.vector.tensor_scalar(y2s_cols[:, t:t + 1], t1[:], m_d[:],
                                        1.0 / 127.0, Alu.mult, Alu.mult)
            ps4_ctx.__exit__(None, None, None)
            qs_bc = cols_to_row_bcast(qs_cols, "qs")

            # ---- P4b + P5 ----
            with tc.tile_pool(name="q2p", bufs=1) as q2p:
                q2T = q2p.tile([128, IK * TPC], BF16)
                with tc.tile_pool(name="p4b", bufs=3) as p4b:
                    for j in range(IK):
                        hj = p4b.tile([128, TPC], F32, tag="h4", name=f"h4{j}")
                        nc.sync.dma_start(hj[:],
                                          h_dram[j * 128:(j + 1) * 128, :])
                        hg2 = p4b.tile([128, TPC], F32, tag="hg2",
                                       name=f"g4{j}")
                        nc.vector.tensor_scalar_mul(hg2[:], hj[:],
                                                    gdc_sb[:, j:j + 1])
                        nc.vector.tensor_tensor(hg2[:], hg2[:], qs_bc[:],
                                                Alu.mult)
                        nc.vector.tensor_scalar(hg2[:], hg2[:], C_MAGIC,
                                                C_MAGIC, Alu.add, Alu.subtract)
                        nc.vector.tensor_copy(q2T[:, j * TPC:(j + 1) * TPC],
                                              hg2[:])

                with tc.tile_pool(name="p5", bufs=4) as p5, \
                     tc.tile_pool(name="psMM2", bufs=2, space="PSUM") as psM2:
                    for hc in range(4):
                        p2 = [psM2.tile([128, 512], F32, tag=f"p2{t}",
                                        name=f"p2_{hc}_{t}") for t in range(TT)]
                        for j in range(IK):
                            td_j = p5.tile([128, 512], BF16, tag="td",
                                           name=f"td{hc}_{j}")
                            nc.sync.dma_start(
                                td_j[:], td_full[j * 128:(j + 1) * 128,
                                                 hc * 512:(hc + 1) * 512])
                            st, sp = j == 0, j == IK - 1
                            for t in range(TT):
                                nc.tensor.matmul(
                                    p2[t][:],
                                    q2T[:, j * TPC + t * 128:
                                        j * TPC + (t + 1) * 128],
                                    td_j[:], start=st, stop=sp)
                        for t in range(TT):
                            yt = p5.tile([128, 512], F32, tag="yt",
                                         name=f"yt{hc}_{t}")
                            nc.vector.tensor_scalar_mul(yt[:], p2[t][:],
                                                        y2s_cols[:, t:t + 1])
                            nc.sync.dma_start(
                                y_ap[t * 128:(t + 1) * 128,
                                     hc * 512:(hc + 1) * 512], yt[:])
    return nc


_CACHE = {}


def _get_compiled():
    if "nc" not in _CACHE:
        nc = bacc.Bacc("TRN2", target_bir_lowering=False, debug=False,
                       enable_asserts=False, num_devices=NC_N)
        build(nc)
        nc.compile()
        _CACHE["nc"] = nc
    return _CACHE["nc"]


def kernel(x, w_gate, g_gate, w_down, g_down):
    nc = _get_compiled()
    x2 = np.ascontiguousarray(np.asarray(x, np.float32).reshape(TOK, H))
    wgT = np.asarray(w_gate, np.float32).T
    wdT = np.asarray(w_down, np.float32).T
    gg = np.ascontiguousarray(np.asarray(g_gate, np.float32).reshape(1, H))
    gdc = np.ascontiguousarray(
        np.asarray(g_down, np.float32).reshape(IK, 128).T)
    in_maps = []
    for c in range(NC_N):
        in_maps.append({
            "x": x2[c * TPC:(c + 1) * TPC],
            "wgt": np.ascontiguousarray(wgT[c * GSH:(c + 1) * GSH]),
            "wdt": np.ascontiguousarray(wdT[c * DSH:(c + 1) * DSH]),
            "gg": gg,
            "gdc": gdc,
        })
    res = run_bass_kernel_spmd(nc, in_maps, core_ids=list(range(NC_N)))
    out = np.concatenate([res.results[c]["y"] for c in range(NC_N)], axis=0)
    return out.reshape(B, S, H).astype(np.float32)

